# revision 1
# baseline (speedup 1.0000x reference)
"""BitLinear (RMSNorm + per-token int8 act fake-quant + ternary weight fake-quant
+ linear) Trainium2 Bass kernel, data-parallel over 8 NeuronCores.

Strategy
--------
Tokens (B*S = 32768) are sharded 8 ways (4096 tokens/core); W is replicated.
Host prep: the global weight-quant scale (one scalar statistic) and the
ternary weight quantization are computed on host with fp32 semantics matching
the reference; the ternary weights ship as fp8e4 (exact for {-1,0,1}, half
the DMA bytes) in k-major layout (the PE contracts over the partition dim of
both matmul operands).

Per core, per 128-token tile [128, 2048]:
  ACT:  sumsq via Square+accum_out -> rms = 1/sqrt(mean+eps) (Sqrt + DVE recip)
  DVE:  xn = (x * rms) * gamma  (scalar_tensor_tensor, exact reference order)
        absmax(xn) -> a = 127/(max+eps)
  ACT:  y = xn*a + C  (fma + magic constant C=1.5*2^23 gives exact RNE round)
  DVE:  q = y - C  -> bf16 ints in [-127, 127] (exactly representable)
  DMA:  xbar transpose q -> qT [k-part, kb, t]  (keeps the PE free)
  PE:   16 k-blocks x 4 out-groups matmuls, bf16 x fp8 (integer-exact, fp32
        psum; one PSUM bank per out-group, kt-major so weights load once per
        k-block and 4 matmuls reuse them)
  ACT:  out = psum * (1/(a*w_scale))  -> f32, DMA out

The matmul is numerically exact: quantized activations are integers
|I|<=127 (bf16-exact) and weights are ternary (fp8-exact), so products and
fp32 partial sums (<2^24) carry no rounding error.  The modeled time is
~478us/core: PE-bound at ~447us of back-to-back N=512 matmuls (the bf16
1 col/cycle roofline is 437us) plus ~22us pipeline fill and ~5us tail.
"""
import numpy as np
from contextlib import ExitStack

import concourse.bacc as bacc
import concourse.tile as tile
from concourse import mybir
from concourse.bass_utils import run_bass_kernel_spmd

F32 = mybir.dt.float32
BF16 = mybir.dt.bfloat16
FP8 = mybir.dt.float8e4
AL = mybir.AluOpType
AF = mybir.ActivationFunctionType
AX = mybir.AxisListType

B, S, DIN, DOUT = 4, 8192, 2048, 2048
NCORES = 8
TOK = B * S                  # 32768
TPC = TOK // NCORES          # 4096 tokens per core
NT = TPC // 128              # 32 token tiles per core
KB = DIN // 128              # 16 contraction blocks
OGW = 512                    # psum free dim per matmul
OG = DOUT // OGW             # 4 output groups

C_MAGIC = 12582912.0         # 1.5 * 2^23: fp32 +C/-C rounds to nearest int (RNE)
WCLIP = 1.4999999            # clip-before-round == round-then-clip to [-1,1]

_CACHE = {}


def _build():
    nc = bacc.Bacc("TRN2", target_bir_lowering=False, debug=False,
                   num_devices=NCORES)
    x_d = nc.declare_dram_parameter("x", [TPC, DIN], F32, isOutput=False)
    g_d = nc.declare_dram_parameter("gamma", [1, DIN], F32, isOutput=False)
    wq_d = nc.declare_dram_parameter("wq", [DIN, DOUT], FP8, isOutput=False)
    sc_d = nc.declare_dram_parameter("sc", [1, 1], F32, isOutput=False)
    o_d = nc.declare_dram_parameter("out", [TPC, DOUT], F32, isOutput=True)

    with tile.TileContext(nc) as tc:
        with ExitStack() as ctx:
            cst = ctx.enter_context(tc.tile_pool(name="cst", bufs=1))
            wqp = ctx.enter_context(tc.tile_pool(name="wqp", bufs=1))
            xp = ctx.enter_context(tc.tile_pool(name="xp", bufs=4))
            sp = ctx.enter_context(tc.tile_pool(name="sp", bufs=3))
            qp = ctx.enter_context(tc.tile_pool(name="qp", bufs=3))
            qtp = ctx.enter_context(tc.tile_pool(name="qtp", bufs=3))
            op = ctx.enter_context(tc.tile_pool(name="op", bufs=3))
            st = ctx.enter_context(tc.tile_pool(name="st", bufs=4))
            pso = ctx.enter_context(tc.tile_pool(name="pso", bufs=2, space="PSUM"))

            # ---- constants + DMA issue order ----
            # The DMA engine pool drains roughly in issue order, so prioritize:
            # first x tile, gamma (needed by the first normalize), then the
            # ternary weight chunks (needed progressively by the first tile's
            # k-block matmuls), then the x-tile stream.
            NPRE = 2  # x tiles DMA'd ahead of the weight chunks
            xpre = [xp.tile([128, DIN], F32, name="xt", tag="xtile")
                    for _ in range(NPRE)]
            nc.sync.dma_start(out=xpre[0], in_=x_d[0:128, :])
            gam = cst.tile([128, DIN], F32, name="gam")
            nc.sync.dma_start(out=gam, in_=g_d[:].to_broadcast((128, DIN)))
            scb = cst.tile([128, 1], F32, name="scb")
            nc.sync.dma_start(out=scb, in_=sc_d[:].to_broadcast((128, 1)))
            inv_b = scb[:, 0:1]    # 1/w_scale
            cmag = cst.tile([128, 1], F32, name="cmag")
            nc.vector.memset(cmag, C_MAGIC)
            ceps = cst.tile([128, 1], F32, name="ceps")
            nc.vector.memset(ceps, 1e-6)
            warmt = cst.tile([128, 1], F32, name="warmt")
            nc.scalar.activation(out=warmt, in_=cmag, func=AF.Square)
            nc.scalar.activation(out=warmt, in_=cmag, func=AF.Sqrt)

            for it in range(1, NPRE):
                nc.sync.dma_start(out=xpre[it],
                                  in_=x_d[it * 128:(it + 1) * 128, :])

            # ---- ternary weights (host-quantized fp8, exact for
            # {-1,0,1}), k-major in SBUF; chunked per k-block and issued in
            # two groups so tile 0's transpose DMA isn't queued behind them ----
            wq = wqp.tile([128, KB, DOUT], FP8, name="wq")

            def dma_wq(kt):
                nc.sync.dma_start(out=wq[:, kt, :],
                                  in_=wq_d[kt * 128:(kt + 1) * 128, :])
            for kt in range(3):
                dma_wq(kt)

            # ---- token tiles ----
            for it in range(NT):
                split = 1
                HW_ = DIN // split
                parts = [(h * HW_, HW_) for h in range(split)]

                if it < NPRE:
                    xt = xpre[it]
                else:
                    xt = xp.tile([128, DIN], F32, name="xt", tag="xtile")
                    nc.sync.dma_start(out=xt,
                                      in_=x_d[it * 128:(it + 1) * 128, :])

                # sum of squares of raw x (per token)
                scr = sp.tile([128, DIN], F32, name="scr")
                sshs = []
                for h, (o0, w) in enumerate(parts):
                    ssh = st.tile([128, 1], F32, name=f"ss{h}", tag=f"ss{h}")
                    nc.scalar.activation(out=scr[:, o0:o0 + w],
                                         in_=xt[:, o0:o0 + w], func=AF.Square,
                                         accum_out=ssh)
                    sshs.append(ssh)
                ss = sshs[0]
                if split > 1:
                    ss = st.tile([128, 1], F32, name="ss")
                    nc.vector.tensor_tensor(out=ss, in0=sshs[0], in1=sshs[1],
                                            op=AL.add)
                # rms = 1/sqrt(ss/DIN + 1e-6)
                sqv = st.tile([128, 1], F32, name="sqv")
                nc.scalar.activation(out=sqv, in_=ss, func=AF.Sqrt, bias=ceps,
                                     scale=1.0 / DIN)
                rms = st.tile([128, 1], F32, name="rms")
                nc.vector.reciprocal(out=rms, in_=sqv)

                # xn = (x * rms) * gamma   (in-place, exact reference order)
                mxhs = []
                for h, (o0, w) in enumerate(parts):
                    nc.vector.scalar_tensor_tensor(
                        out=xt[:, o0:o0 + w], in0=xt[:, o0:o0 + w], scalar=rms,
                        in1=gam[:, o0:o0 + w], op0=AL.mult, op1=AL.mult)
                    mxh = st.tile([128, 1], F32, name=f"mx{h}", tag=f"mx{h}")
                    nc.vector.reduce_max(out=mxh, in_=xt[:, o0:o0 + w],
                                         axis=AX.X, apply_absolute_value=True)
                    mxhs.append(mxh)
                mx = mxhs[0]
                if split > 1:
                    mx = st.tile([128, 1], F32, name="mx")
                    nc.vector.tensor_tensor(out=mx, in0=mxhs[0], in1=mxhs[1],
                                            op=AL.max)

                # a = 127 / (absmax(xn) + 1e-5)
                d = st.tile([128, 1], F32, name="d")
                nc.vector.tensor_scalar(out=d, in0=mx, scalar1=1e-5,
                                        scalar2=None, op0=AL.add)
                rcd = st.tile([128, 1], F32, name="rcd")
                nc.vector.reciprocal(out=rcd, in_=d)
                a = st.tile([128, 1], F32, name="a")
                nc.vector.tensor_scalar(out=a, in0=rcd, scalar1=127.0,
                                        scalar2=None, op0=AL.mult)
                # s3 = (1/a) * (1/ws)
                ra = st.tile([128, 1], F32, name="ra")
                nc.vector.reciprocal(out=ra, in_=a)
                s3 = st.tile([128, 1], F32, name="s3")
                nc.vector.tensor_scalar(out=s3, in0=ra, scalar1=inv_b,
                                        scalar2=None, op0=AL.mult)

                # y = xn*a + C (ACT fma: exact RNE round); q = y - C -> bf16;
                # transpose to contraction-major via the DMA xbar engine
                # (keeps the PE free; separate queue family from bulk copies)
                q = qp.tile([128, DIN], BF16, name="q")
                qt = qtp.tile([128, KB, 128], BF16, name="qt")
                for h, (o0, w) in enumerate(parts):
                    nc.scalar.activation(out=scr[:, o0:o0 + w],
                                         in_=xt[:, o0:o0 + w],
                                         func=AF.Identity, bias=cmag, scale=a)
                    nc.vector.tensor_scalar(out=q[:, o0:o0 + w],
                                            in0=scr[:, o0:o0 + w],
                                            scalar1=C_MAGIC, scalar2=None,
                                            op0=AL.subtract)
                    kb0, kbw = o0 // 128, w // 128
                    nc.scalar.dma_start_transpose(qt[:, kb0:kb0 + kbw, :],
                                                  q[:, o0:o0 + w])
                if it == 0:
                    for kt in range(3, KB):
                        dma_wq(kt)

                # matmul: out[t, o] = sum_k qT[k, t] * wq[k, o]
                # last tile runs og-major so its evacuation overlaps the MMs
                pos = [pso.tile([128, OGW], F32, name=f"po{og}", tag=f"po{og}")
                       for og in range(OG)]
                ot = op.tile([128, DOUT], F32, name="ot")
                last = it == NT - 1
                if True:
                    for kt in range(KB):
                        lhsT = qt[:, kt, :]
                        for og in range(OG):
                            nc.tensor.matmul(
                                pos[og], lhsT=lhsT,
                                rhs=wq[:, kt, og * OGW:(og + 1) * OGW],
                                start=(kt == 0), stop=(kt == KB - 1))
                    for og in range(OG):
                        nc.scalar.mul(out=ot[:, og * OGW:(og + 1) * OGW],
                                      in_=pos[og], mul=s3)
                        if last:
                            nc.sync.dma_start(
                                out=o_d[it * 128:(it + 1) * 128,
                                        og * OGW:(og + 1) * OGW],
                                in_=ot[:, og * OGW:(og + 1) * OGW])
                    if not last:
                        nc.sync.dma_start(out=o_d[it * 128:(it + 1) * 128, :],
                                          in_=ot)

    nc.compile()
    return nc


def kernel(x, gamma, W):
    x = np.asarray(x, dtype=np.float32)
    gamma = np.asarray(gamma, dtype=np.float32)
    W = np.asarray(W, dtype=np.float32)

    # host prep: ternary-quantized k-major weights + the global scale, using
    # fp32 semantics matching the reference:
    #   w_scale = 1/(mean|W| + 1e-5);  w_q = clip(round(W*w_scale), -1, 1)
    import ml_dtypes
    m = np.float32(np.abs(W).astype(np.float64).mean())
    denom = np.float32(m + np.float32(1e-5))
    ws = np.float32(np.float32(1.0) / denom)
    inv_ws = np.float32(np.float32(1.0) / ws)
    wqh = np.clip(np.rint((W * ws).astype(np.float32)), -1.0, 1.0)
    wq = np.ascontiguousarray(wqh.T.astype(ml_dtypes.float8_e4m3))
    sc = np.array([[inv_ws]], dtype=np.float32)
    g2 = gamma.reshape(1, DIN)

    if "nc" not in _CACHE:
        _CACHE["nc"] = _build()
    nc = _CACHE["nc"]

    xf = x.reshape(TOK, DIN)
    in_maps = [
        {"x": xf[c * TPC:(c + 1) * TPC], "gamma": g2, "wq": wq, "sc": sc}
        for c in range(NCORES)
    ]
    res = run_bass_kernel_spmd(nc, in_maps, list(range(NCORES)))
    out = np.concatenate([res.results[c]["out"] for c in range(NCORES)], axis=0)
    return out.reshape(B, S, DOUT)


if __name__ == "__main__":
    rng = np.random.default_rng(0)
    x = rng.standard_normal((B, S, DIN), dtype=np.float32)
    gamma = np.ones((DIN,), dtype=np.float32)
    bound = 1.0 / np.sqrt(DIN)
    W = rng.uniform(-bound, bound, (DOUT, DIN)).astype(np.float32)
    out = kernel(x, gamma, W)
    print("out", out.shape, out.dtype, float(np.abs(out).mean()))



# revision 2
# speedup vs baseline: 1.1570x; 1.1570x over previous
"""BitLinear (RMSNorm + per-token int8 act fake-quant + ternary weight fake-quant
+ linear) Trainium2 Bass kernel, data-parallel over 8 NeuronCores.

Strategy
--------
Tokens (B*S = 32768) are sharded 8 ways (4096 tokens/core); W is replicated.
Host prep: ternary weight quantization (per-tensor stat, fp32 semantics
matching the reference) ships as fp8e4 DUPLICATED pairs (-16*w, w) in k-major
layout; x ships as bf16 (halves input DMA; ~0.3% effect well inside the 2e-2
gate); the output returns as bf16 and is upcast on host (same rationale).

The matmul runs in fp8 DoubleRow mode (2 MACs/cell/cycle) while staying
integer-exact via a hi/lo split of the int8 activations:
    q = round(xn*a) in [-127,127];  hneg = -round(q/16);  l = q + 16*hneg
    q*w = hneg*(-16w) + l*w         (all of hneg, l, -16w, w fp8e4-exact)
Each DoubleRow matmul contracts the (hi,lo) pair per k: per 128-token tile,
16 k-blocks x 4 out-groups x 512 cols at 0.5 cycles/col = 16384 PE cycles,
half of the bf16 rate. Verified bit-exact on hardware (mm_test.py).

Per core, per 128-token tile [128, 2048] (engines balanced under the 6.83us
PE tile time):
  ACT :  sumsq via Square+accum, sqrt, u = x*s + C (magic RNE round),
         psum evacuation * s3 -> bf16
  DVE :  absmax, per-token scalars (rms, a, s=rms*a, s3=1/(a*ws)),
         q = u - C -> bf16, l = (hneg*16) + qt -> fp8
  DMA :  xbar-transpose q -> qt [k, kt, t] (bf16)
  Pool:  t2 = qt*(-1/16) - 192 (bf16 magic: ulp=1 in [128,256)),
         hneg = t2 + 192 -> fp8
  PE  :  64 DoubleRow matmuls, fp8 x fp8, fp32 psum (integer-exact)

Modeled ~225us/core: PE-bound at 218.5us of back-to-back DoubleRow matmuls
plus pipeline fill; DMA total ~174us (46.6 in + 57 transpose + 46.6 out +
23 weights) overlaps under the PE roofline.
"""
import numpy as np
from contextlib import ExitStack

import concourse.bacc as bacc
import concourse.tile as tile
from concourse import mybir
from concourse.bass_utils import run_bass_kernel_spmd

F32 = mybir.dt.float32
BF16 = mybir.dt.bfloat16
FP8 = mybir.dt.float8e4
AL = mybir.AluOpType
AF = mybir.ActivationFunctionType
AX = mybir.AxisListType
PM = mybir.MatmulPerfMode.DoubleRow

B, S, DIN, DOUT = 4, 8192, 2048, 2048
NCORES = 8
TOK = B * S                  # 32768
TPC = TOK // NCORES          # 4096 tokens per core
NT = TPC // 128              # 32 token tiles per core
KB = DIN // 128              # 16 contraction blocks
OGW = 512                    # psum free dim per matmul
OG = DOUT // OGW             # 4 output groups

C_MAGIC = 12582912.0         # 1.5 * 2^23: fp32 +C/-C rounds to nearest int (RNE)
C_BF16 = 192.0               # bf16 magic for h: ulp=1 in [128,256) (8-bit significand)

_CACHE = {}


def _build():
    nc = bacc.Bacc("TRN2", target_bir_lowering=False, debug=False,
                   num_devices=NCORES)
    x_d = nc.declare_dram_parameter("x", [TPC, DIN], BF16, isOutput=False)
    w2_d = nc.declare_dram_parameter("w2", [DIN, 2 * DOUT], FP8, isOutput=False)
    sc_d = nc.declare_dram_parameter("sc", [1, 1], F32, isOutput=False)
    o_d = nc.declare_dram_parameter("out", [TPC, DOUT], BF16, isOutput=True)

    with tile.TileContext(nc) as tc:
        with ExitStack() as ctx:
            cst = ctx.enter_context(tc.tile_pool(name="cst", bufs=1))
            w2p = ctx.enter_context(tc.tile_pool(name="w2p", bufs=1))
            xp = ctx.enter_context(tc.tile_pool(name="xp", bufs=4))
            up = ctx.enter_context(tc.tile_pool(name="up", bufs=2))
            qp = ctx.enter_context(tc.tile_pool(name="qp", bufs=3))
            qtp = ctx.enter_context(tc.tile_pool(name="qtp", bufs=3))
            t2p = ctx.enter_context(tc.tile_pool(name="t2p", bufs=3))
            hlp = ctx.enter_context(tc.tile_pool(name="hlp", bufs=3))
            op = ctx.enter_context(tc.tile_pool(name="op", bufs=3))
            st = ctx.enter_context(tc.tile_pool(name="st", bufs=4))
            pso = ctx.enter_context(tc.tile_pool(name="pso", bufs=2, space="PSUM"))

            # ---- DMA issue order: first x tiles, the scale, then the first
            # weight chunks; the rest of the weights go after tile 0's
            # transpose so its quant pipeline isn't starved.
            NPRE = 2
            xpre = [xp.tile([128, DIN], BF16, name="xt", tag="xtile")
                    for _ in range(NPRE)]
            nc.sync.dma_start(out=xpre[0], in_=x_d[0:128, :])
            scb = cst.tile([128, 1], F32, name="scb")
            nc.sync.dma_start(out=scb, in_=sc_d[:].to_broadcast((128, 1)))
            inv127 = scb[:, 0:1]   # 1/(127*w_scale)
            cmag = cst.tile([128, 1], F32, name="cmag")
            nc.vector.memset(cmag, C_MAGIC)
            ceps = cst.tile([128, 1], F32, name="ceps")
            nc.vector.memset(ceps, 1e-6)
            warmt = cst.tile([128, 1], F32, name="warmt")
            nc.scalar.activation(out=warmt, in_=cmag, func=AF.Square)
            nc.scalar.activation(out=warmt, in_=cmag, func=AF.Sqrt)

            for it in range(1, NPRE):
                nc.sync.dma_start(out=xpre[it],
                                  in_=x_d[it * 128:(it + 1) * 128, :])

            # ---- weight pairs (-16w, w), fp8 k-major, [128, kt, 2, DOUT]
            w2 = w2p.tile([128, KB, 2, DOUT], FP8, name="w2")

            def dma_w2(kt):
                nc.sync.dma_start(out=w2[:, kt, :, :],
                                  in_=w2_d[kt * 128:(kt + 1) * 128, :])
            for kt in range(3):
                dma_w2(kt)

            # ---- token tiles ----
            for it in range(NT):
                if it < NPRE:
                    xt = xpre[it]
                else:
                    xt = xp.tile([128, DIN], BF16, name="xt", tag="xtile")
                    nc.sync.dma_start(out=xt,
                                      in_=x_d[it * 128:(it + 1) * 128, :])

                # per-token stats on raw x: sumsq (ACT) and absmax (DVE)
                scr = up.tile([128, DIN], F32, name="scr")
                ss = st.tile([128, 1], F32, name="ss")
                nc.scalar.activation(out=scr, in_=xt, func=AF.Square,
                                     accum_out=ss)
                mx = st.tile([128, 1], F32, name="mx")
                nc.vector.tensor_reduce(out=mx, in_=xt, axis=AX.X, op=AL.max,
                                        apply_absolute_value=True)
                # rms = 1/sqrt(ss/DIN + 1e-6)
                sqv = st.tile([128, 1], F32, name="sqv")
                nc.scalar.activation(out=sqv, in_=ss, func=AF.Sqrt, bias=ceps,
                                     scale=1.0 / DIN)
                rms = st.tile([128, 1], F32, name="rms")
                nc.vector.reciprocal(out=rms, in_=sqv)
                # d = rms*mx + 1e-5;  s = rms*127/d;  s3 = d/(127*ws)
                mxn = st.tile([128, 1], F32, name="mxn")
                nc.vector.tensor_tensor(out=mxn, in0=mx, in1=rms, op=AL.mult)
                d = st.tile([128, 1], F32, name="d")
                nc.vector.tensor_scalar(out=d, in0=mxn, scalar1=1e-5,
                                        scalar2=None, op0=AL.add)
                rcd = st.tile([128, 1], F32, name="rcd")
                nc.vector.reciprocal(out=rcd, in_=d)
                s = st.tile([128, 1], F32, name="s")
                nc.vector.scalar_tensor_tensor(out=s, in0=rcd, scalar=127.0,
                                               in1=rms, op0=AL.mult,
                                               op1=AL.mult)
                s3 = st.tile([128, 1], F32, name="s3")
                nc.vector.tensor_scalar(out=s3, in0=d, scalar1=inv127,
                                        scalar2=None, op0=AL.mult)

                # q = RNE(x*s) via fp32 magic (ACT fma + DVE unbias)
                nc.scalar.activation(out=scr, in_=xt, func=AF.Identity,
                                     bias=cmag, scale=s)
                q = qp.tile([128, DIN], BF16, name="q")
                nc.vector.tensor_scalar(out=q, in0=scr, scalar1=C_MAGIC,
                                        scalar2=None, op0=AL.subtract)

                # transpose to contraction-major (DMA xbar, bf16)
                qt = qtp.tile([128, KB, 128], BF16, name="qt")
                nc.scalar.dma_start_transpose(qt, q)
                if it == 0:
                    for kt in range(3, KB):
                        dma_w2(kt)

                # hi/lo split in k-major space:
                #   t2 = -q/16 - 192 (bf16 RNE at ulp=1) -> hneg = t2 + 192
                #   l  = q + 16*hneg
                t2 = t2p.tile([128, KB, 128], BF16, name="t2")
                nc.gpsimd.tensor_scalar(out=t2, in0=qt, scalar1=-1.0 / 16.0,
                                        scalar2=-C_BF16, op0=AL.mult,
                                        op1=AL.add)
                qhl = hlp.tile([128, KB, 2, 128], FP8, name="qhl")
                nc.gpsimd.tensor_scalar(out=qhl[:, :, 0, :], in0=t2,
                                        scalar1=C_BF16, scalar2=None,
                                        op0=AL.add)
                nc.vector.scalar_tensor_tensor(out=qhl[:, :, 1, :],
                                               in0=qhl[:, :, 0, :],
                                               scalar=16.0, in1=qt,
                                               op0=AL.mult, op1=AL.add)

                # matmul: out[t, o] = sum_k sum_j qhl[k, j, t] * w2[k, j, o]
                pos = [pso.tile([128, OGW], F32, name=f"po{og}", tag=f"po{og}")
                       for og in range(OG)]
                ot = op.tile([128, DOUT], BF16, name="ot")
                last = it == NT - 1
                for kt in range(KB):
                    lhsT = qhl[:, kt, :, :]
                    for og in range(OG):
                        nc.tensor.matmul(
                            pos[og], lhsT=lhsT,
                            rhs=w2[:, kt, :, og * OGW:(og + 1) * OGW],
                            start=(kt == 0), stop=(kt == KB - 1),
                            perf_mode=PM)
                for og in range(OG):
                    nc.scalar.mul(out=ot[:, og * OGW:(og + 1) * OGW],
                                  in_=pos[og], mul=s3)
                    if last:
                        nc.sync.dma_start(
                            out=o_d[it * 128:(it + 1) * 128,
                                    og * OGW:(og + 1) * OGW],
                            in_=ot[:, og * OGW:(og + 1) * OGW])
                if not last:
                    nc.sync.dma_start(out=o_d[it * 128:(it + 1) * 128, :],
                                      in_=ot)

    nc.compile()
    return nc


def kernel(x, gamma, W):
    import ml_dtypes

    x = np.asarray(x, dtype=np.float32)
    gamma = np.asarray(gamma, dtype=np.float32)
    W = np.asarray(W, dtype=np.float32)

    # host prep: ternary weight pairs + the global scale, fp32 semantics
    # matching the reference: w_scale = 1/(mean|W| + 1e-5)
    m = np.float32(np.abs(W).astype(np.float64).mean())
    denom = np.float32(m + np.float32(1e-5))
    ws = np.float32(np.float32(1.0) / denom)
    wqh = np.clip(np.rint((W * ws).astype(np.float32)), -1.0, 1.0)
    wt = np.ascontiguousarray(wqh.T)                      # [DIN, DOUT]
    w2 = np.empty((DIN, 2, DOUT), dtype=ml_dtypes.float8_e4m3)
    w2[:, 0, :] = (-16.0 * wt).astype(ml_dtypes.float8_e4m3)
    w2[:, 1, :] = wt.astype(ml_dtypes.float8_e4m3)
    w2 = w2.reshape(DIN, 2 * DOUT)
    sc = np.array([[1.0 / (127.0 * float(ws))]], dtype=np.float32)

    if not np.all(gamma == 1.0):
        x = x * gamma  # reference order is (x*rms)*gamma; ~1ulp difference
    xb = x.reshape(TOK, DIN).astype(ml_dtypes.bfloat16)

    if "nc" not in _CACHE:
        _CACHE["nc"] = _build()
    nc = _CACHE["nc"]

    in_maps = [
        {"x": xb[c * TPC:(c + 1) * TPC], "w2": w2, "sc": sc}
        for c in range(NCORES)
    ]
    res = run_bass_kernel_spmd(nc, in_maps, list(range(NCORES)))
    out = np.concatenate([res.results[c]["out"] for c in range(NCORES)],
                         axis=0)
    return out.astype(np.float32).reshape(B, S, DOUT)


if __name__ == "__main__":
    rng = np.random.default_rng(0)
    x = rng.standard_normal((B, S, DIN), dtype=np.float32)
    gamma = np.ones((DIN,), dtype=np.float32)
    bound = 1.0 / np.sqrt(DIN)
    W = rng.uniform(-bound, bound, (DOUT, DIN)).astype(np.float32)
    out = kernel(x, gamma, W)
    print("out", out.shape, out.dtype, float(np.abs(out).mean()))
